# revision 13
# baseline (speedup 1.0000x reference)
"""Trainium2 Bass kernel for DiscreteSSL vq_codebook (argmin-VQ + codebook projection).

Computation (per layer l, batch b):
    x = h[l,b] viewed as [D, S] (already transposed in input layout)
    scores[s,k] = x[:,s] . centers[l,k] - 0.5*||centers[l,k]||^2
    tokens = argmax_k scores                       (== argmin of sq-distance)
    embs   = (centers[l] @ W[l] + b[l])[tokens]    (gather from precomputed G)

Sharding: 8 cores = (layer l in 0..3) x (batch half in 0..1); each core
handles 8 batches of one layer. centers/W/b replicated per-layer.

SCORE_MODE:
  fp32  - true fp32 matmuls (4 cyc/row on PE).
  f32r3 - 3-pass hi/lo FP32R split at 3 cyc/row:
            x ~ xhi + xlo, c ~ chi + clo (each FP32R, 11-bit mantissa)
            scores = xhi@chi + xhi@clo + xlo@chi     (xlo@clo ~ 2^-24, dropped)
          All products of 11-bit operands are exact in the PE's FP22 datapath +
          fp32 accumulator; max |error| vs fp64 is ~9e-5 on the actual data
          (min top-2 gap is 3.7e-4) -> 0 token flips, verified on HW.
          NOTE: bf16 matmuls must NOT be mixed into f32r accumulation groups --
          that corrupts results on HW (observed 17.6%% wrong argmaxes).
"""

import os
import numpy as np

L, B, D, T, F = 4, 16, 768, 125, 8
S = T * F          # 1000 frames
K = 1000           # codebook size
KT = 512           # target dim
NCORES = 8
BH = B // 2        # batches per core
P = 128
NCH = D // P       # 6 contraction chunks

SCORE_MODE = os.environ.get("SCORE_MODE", "f32r3")

_programs = {}
_runners = {}


def _round_f32r(x):
    """Round fp32 -> FP32R (11-bit mantissa, RNE, low 12 bits zeroed)."""
    u = np.ascontiguousarray(x, np.float32).view(np.uint32)
    r = (u + np.uint32(0x7FF) + ((u >> np.uint32(12)) & np.uint32(1))) & np.uint32(0xFFFFF000)
    return r.view(np.float32)


def _build_program(score_mode=SCORE_MODE, reps=1):
    import concourse.tile as tile
    from concourse import bacc, mybir
    from concourse.bass import IndirectOffsetOnAxis
    from contextlib import ExitStack

    dt = mybir.dt
    nc = bacc.Bacc("TRN2", target_bir_lowering=False, debug=False,
                   enable_partition_id=False)

    f32r_mode = score_mode == "f32r3"
    if f32r_mode:
        hT_hi = nc.dram_tensor("hT_hi", [BH, D, S], dt.float32r, kind="ExternalInput")
        hT_lo = nc.dram_tensor("hT_lo", [BH, D, S], dt.float32r, kind="ExternalInput")
        cT_hi = nc.dram_tensor("cT_hi", [D, K], dt.float32r, kind="ExternalInput")
        cT_lo = nc.dram_tensor("cT_lo", [D, K], dt.float32r, kind="ExternalInput")
        w_hi = nc.dram_tensor("w_hi", [D, KT], dt.float32r, kind="ExternalInput")
        w_lo = nc.dram_tensor("w_lo", [D, KT], dt.float32r, kind="ExternalInput")
    else:
        hT_f = nc.dram_tensor("hT", [BH, D, S], dt.float32, kind="ExternalInput")
        cT_f = nc.dram_tensor("cT", [D, K], dt.float32, kind="ExternalInput")
        w_f = nc.dram_tensor("w_in", [D, KT], dt.float32, kind="ExternalInput")
    brep = nc.dram_tensor("brep", [P, KT], dt.float32, kind="ExternalInput")

    toks_out = nc.dram_tensor("toks", [BH, S], dt.int32, kind="ExternalOutput")
    embs_out = nc.dram_tensor("embs", [BH, S, KT], dt.float32, kind="ExternalOutput")

    g_dram = nc.dram_tensor("g_dram", [K, KT], dt.float32)

    KT0 = 512                    # first k-tile width
    KT1 = K - KT0                # second k-tile width (488)
    s_tiles = [(i * P, min(P, S - i * P)) for i in range((S + P - 1) // P)]
    m_tiles = [(i * P, min(P, K - i * P)) for i in range((K + P - 1) // P)]

    with ExitStack() as ctx:
        tc = ctx.enter_context(tile.TileContext(nc))
        persist = ctx.enter_context(tc.tile_pool(name="persist", bufs=1))
        sqpool = ctx.enter_context(tc.tile_pool(name="sq", bufs=2))
        gsb_pool = ctx.enter_context(tc.tile_pool(name="gsb", bufs=2))
        prep_psum = ctx.enter_context(tc.tile_pool(name="ppsum", bufs=2, space="PSUM"))
        xt_pool = ctx.enter_context(tc.tile_pool(name="xt", bufs=12))
        main_psum = ctx.enter_context(tc.tile_pool(name="mpsum", bufs=3, space="PSUM"))
        sc_pool = ctx.enter_context(tc.tile_pool(name="sc", bufs=2))
        idx_pool = ctx.enter_context(tc.tile_pool(name="idx", bufs=4))
        emb_pool = ctx.enter_context(tc.tile_pool(name="emb", bufs=3))

        for rep in range(reps):
            sfx = f"_r{rep}" if reps > 1 else ""
            # ---- persistent tiles --------------------------------------------
            if f32r_mode:
                cthi = [persist.tile([P, K], dt.float32r, name=f"cthi{c}{sfx}",
                                     tag=f"cthi{c}") for c in range(NCH)]
                ctlo = [persist.tile([P, K], dt.float32r, name=f"ctlo{c}{sfx}",
                                     tag=f"ctlo{c}") for c in range(NCH)]
                wthi = [persist.tile([P, KT], dt.float32r, name=f"wthi{c}{sfx}",
                                     tag=f"wthi{c}") for c in range(NCH)]
                wtlo = [persist.tile([P, KT], dt.float32r, name=f"wtlo{c}{sfx}",
                                     tag=f"wtlo{c}") for c in range(NCH)]
                for c in range(NCH):
                    sl = slice(c * P, (c + 1) * P)
                    nc.sync.dma_start(out=cthi[c][:], in_=cT_hi[sl, :])
                    nc.sync.dma_start(out=ctlo[c][:], in_=cT_lo[sl, :])
                    nc.sync.dma_start(out=wthi[c][:], in_=w_hi[sl, :])
                    nc.sync.dma_start(out=wtlo[c][:], in_=w_lo[sl, :])
            else:
                ctf = [persist.tile([P, K], dt.float32, name=f"ctf{c}{sfx}",
                                    tag=f"ctf{c}") for c in range(NCH)]
                wt = [persist.tile([P, KT], dt.float32, name=f"w{c}{sfx}",
                                   tag=f"w{c}") for c in range(NCH)]
                for c in range(NCH):
                    sl = slice(c * P, (c + 1) * P)
                    nc.sync.dma_start(out=ctf[c][:], in_=cT_f[sl, :])
                    nc.sync.dma_start(out=wt[c][:], in_=w_f[sl, :])
            brep_sb = persist.tile([P, KT], dt.float32, name=f"brep_sb{sfx}", tag="brep")
            bias_sb = persist.tile([P, K], dt.float32, name=f"bias_sb{sfx}", tag="bias")
            ones_sb = persist.tile([P, P], dt.float32, name=f"ones_sb{sfx}", tag="ones")
            nc.sync.dma_start(out=brep_sb[:], in_=brep[:, :])
            nc.vector.memset(ones_sb[:], 1.0)

            # ---- bias = -0.5 * ||c_k||^2, replicated across partitions -------
            bias_ps = prep_psum.tile([P, KT0], dt.float32, tag="bias_ps", bufs=1,
                                     name=f"bias_ps{sfx}")
            bias_ps2 = prep_psum.tile([P, KT1], dt.float32, tag="bias_ps2", bufs=1,
                                      name=f"bias_ps2{sfx}")
            for c in range(NCH):
                if f32r_mode:
                    tmp = sqpool.tile([P, K], dt.float32, name=f"tmp{c}{sfx}", tag="sq")
                    nc.vector.tensor_add(out=tmp[:], in0=cthi[c][:].bitcast(dt.float32),
                                         in1=ctlo[c][:].bitcast(dt.float32))
                    sq = sqpool.tile([P, K], dt.float32, name=f"sq{c}{sfx}", tag="sq")
                    nc.vector.tensor_mul(out=sq[:], in0=tmp[:], in1=tmp[:])
                else:
                    sq = sqpool.tile([P, K], dt.float32, name=f"sq{c}{sfx}", tag="sq")
                    nc.vector.tensor_mul(out=sq[:], in0=ctf[c][:], in1=ctf[c][:])
                nc.tensor.matmul(out=bias_ps[:], lhsT=ones_sb[:], rhs=sq[:, :KT0],
                                 start=(c == 0), stop=(c == NCH - 1))
                nc.tensor.matmul(out=bias_ps2[:], lhsT=ones_sb[:], rhs=sq[:, KT0:],
                                 start=(c == 0), stop=(c == NCH - 1))
            nc.scalar.activation(out=bias_sb[:, :KT0], in_=bias_ps[:],
                                 func=mybir.ActivationFunctionType.Copy, scale=-0.5)
            nc.scalar.activation(out=bias_sb[:, KT0:], in_=bias_ps2[:],
                                 func=mybir.ActivationFunctionType.Copy, scale=-0.5)

            # ---- G = centers @ W + b  -> g_dram ------------------------------
            for (m0, ms) in m_tiles:
                g_ps_full = main_psum.tile([P, K], dt.float32, tag="ps",
                                           name=f"g_ps{m0}{sfx}")
                g_ps = g_ps_full[:, :KT]
                if f32r_mode:
                    g_passes = [(cthi, wthi), (cthi, wtlo), (ctlo, wthi)]
                else:
                    g_passes = [(ctf, wt)]
                first = True
                for gi, (cs, ws) in enumerate(g_passes):
                    for c in range(NCH):
                        last = gi == len(g_passes) - 1 and c == NCH - 1
                        nc.tensor.matmul(out=g_ps[:ms, :], lhsT=cs[c][:][:, m0:m0 + ms],
                                         rhs=ws[c][:][:, :], start=first, stop=last)
                        first = False
                g_sb = gsb_pool.tile([P, KT], dt.float32, tag="g_sb", name=f"g_sb{m0}{sfx}")
                nc.vector.tensor_add(out=g_sb[:ms, :], in0=g_ps[:ms, :],
                                     in1=brep_sb[:ms, :])
                nc.scalar.dma_start(out=g_dram[m0:m0 + ms, :], in_=g_sb[:ms, :])

            # ---- main loop: scores -> argmax -> gather -----------------------
            for b in range(BH):
                if f32r_mode:
                    xhi, xlo = [], []
                    for c in range(NCH):
                        sl = slice(c * P, (c + 1) * P)
                        th = xt_pool.tile([P, S], dt.float32r, tag="xthi",
                                          name=f"xhi{b}_{c}{sfx}")
                        tl = xt_pool.tile([P, S], dt.float32r, tag="xtlo",
                                          name=f"xlo{b}_{c}{sfx}")
                        nc.sync.dma_start(out=th[:], in_=hT_hi[b, sl, :])
                        nc.sync.dma_start(out=tl[:], in_=hT_lo[b, sl, :])
                        xhi.append(th)
                        xlo.append(tl)
                    passes = [(xhi, cthi), (xhi, ctlo), (xlo, cthi)]
                else:
                    xts = []
                    for c in range(NCH):
                        xt = xt_pool.tile([P, S], dt.float32, tag="xthi",
                                          name=f"xt{b}_{c}{sfx}")
                        nc.sync.dma_start(out=xt[:], in_=hT_f[b, c * P:(c + 1) * P, :])
                        xts.append(xt)
                    passes = [(xts, ctf)]
                np_ = len(passes)
                for (s0, ns) in s_tiles:
                    ps = main_psum.tile([P, K], dt.float32, tag="ps", name=f"ps{b}_{s0}{sfx}")
                    first = True
                    for pi, (xs, cs) in enumerate(passes):
                        last_pass = pi == np_ - 1
                        for c in range(NCH):
                            last = last_pass and c == NCH - 1
                            nc.tensor.matmul(out=ps[:ns, :KT0],
                                             lhsT=xs[c][:][:, s0:s0 + ns],
                                             rhs=cs[c][:][:, :KT0],
                                             start=first, stop=last)
                            nc.tensor.matmul(out=ps[:ns, KT0:],
                                             lhsT=xs[c][:][:, s0:s0 + ns],
                                             rhs=cs[c][:][:, KT0:],
                                             start=first, stop=last)
                            first = False
                    sc = sc_pool.tile([P, K], dt.float32, tag="sc", name=f"sc{b}_{s0}{sfx}")
                    nc.vector.tensor_add(out=sc[:ns, :], in0=ps[:ns, :],
                                         in1=bias_sb[:ns, :])
                    max8 = idx_pool.tile([P, 8], dt.float32, tag="max8",
                                         name=f"max8{b}_{s0}{sfx}")
                    idx8 = idx_pool.tile([P, 8], dt.uint32, tag="idx8",
                                         name=f"idx8{b}_{s0}{sfx}")
                    nc.vector.max(out=max8[:ns, :], in_=sc[:ns, :])
                    nc.vector.max_index(out=idx8[:ns, :], in_max=max8[:ns, :],
                                        in_values=sc[:ns, :])
                    tok = idx_pool.tile([P, 1], dt.int32, tag="tok",
                                        name=f"tok{b}_{s0}{sfx}")
                    nc.vector.tensor_copy(out=tok[:ns, :], in_=idx8[:ns, :1])
                    nc.scalar.dma_start(out=toks_out[b, s0:s0 + ns, None], in_=tok[:ns, :])
                    emb = emb_pool.tile([P, KT], dt.float32, tag="emb", name=f"emb{b}_{s0}{sfx}")
                    nc.gpsimd.indirect_dma_start(
                        out=emb[:ns, :], out_offset=None,
                        in_=g_dram[:, :],
                        in_offset=IndirectOffsetOnAxis(ap=tok[:ns, :1], axis=0),
                    )
                    nc.scalar.dma_start(out=embs_out[b, s0:s0 + ns, :], in_=emb[:ns, :])

    nc.compile()
    return nc


def _get_program(score_mode=SCORE_MODE, reps=1):
    key = (score_mode, reps)
    if key not in _programs:
        _programs[key] = _build_program(score_mode, reps)
    return _programs[key]


def _make_in_maps(h, centers, W, b, score_mode=SCORE_MODE):
    import ml_dtypes
    bf16 = ml_dtypes.bfloat16
    in_maps = []
    for core in range(NCORES):
        l = core // 2
        bh = core % 2
        hT_core = np.ascontiguousarray(
            h[l, bh * BH:(bh + 1) * BH].reshape(BH, D, S)).astype(np.float32, copy=False)
        cT_core = np.ascontiguousarray(centers[l].T).astype(np.float32, copy=False)
        w_core = np.ascontiguousarray(W[l]).astype(np.float32, copy=False)
        brep_core = np.ascontiguousarray(
            np.broadcast_to(b[l][None, :], (P, KT))).astype(np.float32, copy=False)
        m = {"brep": brep_core}
        if score_mode == "f32r3":
            hhi = _round_f32r(hT_core)
            m["hT_hi"] = hhi
            m["hT_lo"] = _round_f32r(hT_core - hhi)
            chi = _round_f32r(cT_core)
            m["cT_hi"] = chi
            m["cT_lo"] = _round_f32r(cT_core - chi)
            whi = _round_f32r(w_core)
            m["w_hi"] = whi
            m["w_lo"] = _round_f32r(w_core - whi)
        else:
            m["hT"] = hT_core
            m["cT"] = cT_core
            m["w_in"] = w_core
        in_maps.append(m)
    return in_maps


class _Runner:
    """Device-resident SPMD runner mirroring bass2jax.run_bass_via_pjrt's
    multi-core path, but reusable across calls (for benchmarking)."""

    def __init__(self, nc, n_cores):
        import jax
        import concourse.mybir as mybir
        from concourse.bass2jax import _bass_exec_p, install_neuronx_cc_hook
        from jax.experimental.shard_map import shard_map
        from jax.sharding import Mesh, PartitionSpec

        install_neuronx_cc_hook()
        self.n_cores = n_cores
        in_names, out_names, out_avals, zero_outs = [], [], [], []
        for alloc in nc.m.functions[0].allocations:
            if not isinstance(alloc, mybir.MemoryLocationSet):
                continue
            name = alloc.memorylocations[0].name
            if alloc.kind == "ExternalInput":
                in_names.append(name)
            elif alloc.kind == "ExternalOutput":
                shape = tuple(alloc.tensor_shape)
                dtype = mybir.dt.np(alloc.dtype)
                out_avals.append(jax.core.ShapedArray(shape, dtype))
                zero_outs.append(np.zeros(shape, dtype))
                out_names.append(name)
        n_params = len(in_names)
        all_in_names = in_names + out_names
        self.in_names, self.out_names = in_names, out_names
        self.out_avals = out_avals

        def _body(*args):
            outs = _bass_exec_p.bind(
                *args,
                out_avals=tuple(out_avals),
                in_names=tuple(all_in_names),
                out_names=tuple(out_names),
                lowering_input_output_aliases=(),
                sim_require_finite=True,
                sim_require_nnan=True,
                nc=nc,
            )
            return tuple(outs)

        devices = jax.devices()[:n_cores]
        assert len(devices) == n_cores
        self.mesh = Mesh(np.asarray(devices), ("core",))
        in_specs = (PartitionSpec("core"),) * (n_params + len(out_names))
        out_specs = (PartitionSpec("core"),) * len(out_names)
        self.fn = jax.jit(
            shard_map(_body, mesh=self.mesh, in_specs=in_specs,
                      out_specs=out_specs, check_rep=False),
            keep_unused=True,
        )
        self.zero_outs = zero_outs
        self._dev_zeros = None

    def put_inputs(self, in_maps):
        import jax
        from jax.sharding import NamedSharding, PartitionSpec
        sh = NamedSharding(self.mesh, PartitionSpec("core"))
        concat_in = [
            np.concatenate([np.asarray(m[name]) for m in in_maps], axis=0)
            for name in self.in_names
        ]
        self._dev_in = [jax.device_put(a, sh) for a in concat_in]
        if self._dev_zeros is None:
            concat_zero = [
                np.zeros((self.n_cores * z.shape[0], *z.shape[1:]), z.dtype)
                for z in self.zero_outs
            ]
            self._dev_zeros = [jax.device_put(a, sh) for a in concat_zero]

    def run(self):
        import jax
        outs = self.fn(*self._dev_in, *self._dev_zeros)
        jax.block_until_ready(outs)
        return outs

    def results(self, outs):
        res = []
        for c in range(self.n_cores):
            res.append({
                name: np.asarray(outs[i]).reshape(
                    self.n_cores, *self.out_avals[i].shape)[c]
                for i, name in enumerate(self.out_names)
            })
        return res

    def timed(self, iters):
        import jax, time
        t0 = time.perf_counter()
        for _ in range(iters):
            outs = self.fn(*self._dev_in, *self._dev_zeros)
        jax.block_until_ready(outs)
        return time.perf_counter() - t0


def _get_runner(score_mode=SCORE_MODE, reps=1):
    key = (score_mode, reps)
    if key not in _runners:
        _runners[key] = _Runner(_get_program(score_mode, reps), NCORES)
    return _runners[key]


def kernel(h, centers, W, b):
    h = np.asarray(h, dtype=np.float32)
    centers = np.asarray(centers, dtype=np.float32)
    W = np.asarray(W, dtype=np.float32)
    b = np.asarray(b, dtype=np.float32)

    runner = _get_runner()
    in_maps = _make_in_maps(h, centers, W, b)
    runner.put_inputs(in_maps)
    outs = runner.run()
    res = runner.results(outs)

    tokens_full = np.empty((B, S, L), np.int32)
    embs_full = np.empty((B, S, L, KT), np.float32)
    for core in range(NCORES):
        l = core // 2
        bh = core % 2
        out = res[core]
        tokens_full[bh * BH:(bh + 1) * BH, :, l] = out["toks"]
        embs_full[bh * BH:(bh + 1) * BH, :, l, :] = out["embs"]
    return tokens_full, embs_full


# revision 15
# speedup vs baseline: 1.2276x; 1.2276x over previous
"""Trainium2 Bass kernel for DiscreteSSL vq_codebook (argmin-VQ + codebook projection).

Computation (per layer l, batch b):
    x = h[l,b] viewed as [D, S] (already transposed in input layout)
    scores[s,k] = x[:,s] . centers[l,k] - 0.5*||centers[l,k]||^2
    tokens = argmax_k scores                       (== argmin of sq-distance)
    embs   = (centers[l] @ W[l] + b[l])[tokens]    (gather from precomputed G)

Sharding: 8 cores = (layer l in 0..3) x (batch half in 0..1); each core
handles 8 batches of one layer. centers/W/b replicated per-layer.

SCORE_MODE:
  fp32  - true fp32 matmuls (4 cyc/row on PE).
  f32r3 - 3-pass hi/lo FP32R split at 3 cyc/row:
            x ~ xhi + xlo, c ~ chi + clo (each FP32R, 11-bit mantissa)
            scores = xhi@chi + xhi@clo + xlo@chi     (xlo@clo ~ 2^-24, dropped)
          All products of 11-bit operands are exact in the PE's FP22 datapath +
          fp32 accumulator; max |error| vs fp64 is ~9e-5 on the actual data
          (min top-2 gap is 3.7e-4) -> 0 token flips, verified on HW.
          NOTE: bf16 matmuls must NOT be mixed into f32r accumulation groups --
          that corrupts results on HW (observed 17.6%% wrong argmaxes).
"""

import os
import numpy as np

L, B, D, T, F = 4, 16, 768, 125, 8
S = T * F          # 1000 frames
K = 1000           # codebook size
KT = 512           # target dim
NCORES = 8
BH = B // 2        # batches per core
P = 128
NCH = D // P       # 6 contraction chunks

SCORE_MODE = os.environ.get("SCORE_MODE", "f32r3")

_programs = {}
_runners = {}


def _round_f32r(x):
    """Round fp32 -> FP32R (11-bit mantissa, RNE, low 12 bits zeroed)."""
    u = np.ascontiguousarray(x, np.float32).view(np.uint32)
    r = (u + np.uint32(0x7FF) + ((u >> np.uint32(12)) & np.uint32(1))) & np.uint32(0xFFFFF000)
    return r.view(np.float32)


def _build_program(score_mode=SCORE_MODE, reps=1):
    import concourse.tile as tile
    from concourse import bacc, mybir
    from concourse.bass import IndirectOffsetOnAxis
    from contextlib import ExitStack

    dt = mybir.dt
    nc = bacc.Bacc("TRN2", target_bir_lowering=False, debug=False,
                   enable_partition_id=False)

    f32r_mode = score_mode == "f32r3"
    if f32r_mode:
        hT_hi = nc.dram_tensor("hT_hi", [BH, D, S], dt.float32r, kind="ExternalInput")
        hT_lo = nc.dram_tensor("hT_lo", [BH, D, S], dt.float32r, kind="ExternalInput")
        cT_hi = nc.dram_tensor("cT_hi", [D, K], dt.float32r, kind="ExternalInput")
        cT_lo = nc.dram_tensor("cT_lo", [D, K], dt.float32r, kind="ExternalInput")
        w_hi = nc.dram_tensor("w_hi", [D, KT], dt.float32r, kind="ExternalInput")
        w_lo = nc.dram_tensor("w_lo", [D, KT], dt.float32r, kind="ExternalInput")
    else:
        hT_f = nc.dram_tensor("hT", [BH, D, S], dt.float32, kind="ExternalInput")
        cT_f = nc.dram_tensor("cT", [D, K], dt.float32, kind="ExternalInput")
        w_f = nc.dram_tensor("w_in", [D, KT], dt.float32, kind="ExternalInput")
    brep = nc.dram_tensor("brep", [P, KT], dt.float32, kind="ExternalInput")

    toks_out = nc.dram_tensor("toks", [BH, S], dt.int32, kind="ExternalOutput")
    embs_out = nc.dram_tensor("embs", [BH, S, KT], dt.float32, kind="ExternalOutput")

    g_dram = nc.dram_tensor("g_dram", [K, KT], dt.float32)

    KT0 = 512                    # first k-tile width
    KT1 = K - KT0                # second k-tile width (488)
    s_tiles = [(i * P, min(P, S - i * P)) for i in range((S + P - 1) // P)]
    m_tiles = [(i * P, min(P, K - i * P)) for i in range((K + P - 1) // P)]

    with ExitStack() as ctx:
        tc = ctx.enter_context(tile.TileContext(nc))
        persist = ctx.enter_context(tc.tile_pool(name="persist", bufs=1))
        sqpool = ctx.enter_context(tc.tile_pool(name="sq", bufs=2))
        gsb_pool = ctx.enter_context(tc.tile_pool(name="gsb", bufs=2))
        prep_psum = ctx.enter_context(tc.tile_pool(name="ppsum", bufs=2, space="PSUM"))
        xt_pool = ctx.enter_context(tc.tile_pool(name="xt", bufs=13))
        main_psum = ctx.enter_context(tc.tile_pool(name="mpsum", bufs=3, space="PSUM"))
        sc_pool = ctx.enter_context(tc.tile_pool(name="sc", bufs=2))
        idx_pool = ctx.enter_context(tc.tile_pool(name="idx", bufs=4))
        emb_pool = ctx.enter_context(tc.tile_pool(name="emb", bufs=3))

        for rep in range(reps):
            sfx = f"_r{rep}" if reps > 1 else ""
            # ---- persistent tiles --------------------------------------------
            if f32r_mode:
                cthi = [persist.tile([P, K], dt.float32r, name=f"cthi{c}{sfx}",
                                     tag=f"cthi{c}") for c in range(NCH)]
                ctlo = [persist.tile([P, K], dt.float32r, name=f"ctlo{c}{sfx}",
                                     tag=f"ctlo{c}") for c in range(NCH)]
                wthi = [persist.tile([P, KT], dt.float32r, name=f"wthi{c}{sfx}",
                                     tag=f"wthi{c}") for c in range(NCH)]
                wtlo = [persist.tile([P, KT], dt.float32r, name=f"wtlo{c}{sfx}",
                                     tag=f"wtlo{c}") for c in range(NCH)]
                for c in range(NCH):
                    sl = slice(c * P, (c + 1) * P)
                    nc.sync.dma_start(out=cthi[c][:], in_=cT_hi[sl, :])
                    nc.sync.dma_start(out=ctlo[c][:], in_=cT_lo[sl, :])
                    nc.sync.dma_start(out=wthi[c][:], in_=w_hi[sl, :])
                    nc.sync.dma_start(out=wtlo[c][:], in_=w_lo[sl, :])
            else:
                ctf = [persist.tile([P, K], dt.float32, name=f"ctf{c}{sfx}",
                                    tag=f"ctf{c}") for c in range(NCH)]
                wt = [persist.tile([P, KT], dt.float32, name=f"w{c}{sfx}",
                                   tag=f"w{c}") for c in range(NCH)]
                for c in range(NCH):
                    sl = slice(c * P, (c + 1) * P)
                    nc.sync.dma_start(out=ctf[c][:], in_=cT_f[sl, :])
                    nc.sync.dma_start(out=wt[c][:], in_=w_f[sl, :])
            brep_sb = persist.tile([P, KT], dt.float32, name=f"brep_sb{sfx}", tag="brep")
            bias_sb = persist.tile([P, K], dt.float32, name=f"bias_sb{sfx}", tag="bias")
            ones_sb = persist.tile([P, P], dt.float32, name=f"ones_sb{sfx}", tag="ones")
            nc.sync.dma_start(out=brep_sb[:], in_=brep[:, :])
            nc.vector.memset(ones_sb[:], 1.0)

            # ---- bias = -0.5 * ||c_k||^2, replicated across partitions -------
            bias_ps = prep_psum.tile([P, KT0], dt.float32, tag="bias_ps", bufs=1,
                                     name=f"bias_ps{sfx}")
            bias_ps2 = prep_psum.tile([P, KT1], dt.float32, tag="bias_ps2", bufs=1,
                                      name=f"bias_ps2{sfx}")
            for c in range(NCH):
                if f32r_mode:
                    tmp = sqpool.tile([P, K], dt.float32, name=f"tmp{c}{sfx}", tag="sq")
                    nc.vector.tensor_add(out=tmp[:], in0=cthi[c][:].bitcast(dt.float32),
                                         in1=ctlo[c][:].bitcast(dt.float32))
                    sq = sqpool.tile([P, K], dt.float32, name=f"sq{c}{sfx}", tag="sq")
                    nc.vector.tensor_mul(out=sq[:], in0=tmp[:], in1=tmp[:])
                else:
                    sq = sqpool.tile([P, K], dt.float32, name=f"sq{c}{sfx}", tag="sq")
                    nc.vector.tensor_mul(out=sq[:], in0=ctf[c][:], in1=ctf[c][:])
                nc.tensor.matmul(out=bias_ps[:], lhsT=ones_sb[:], rhs=sq[:, :KT0],
                                 start=(c == 0), stop=(c == NCH - 1))
                nc.tensor.matmul(out=bias_ps2[:], lhsT=ones_sb[:], rhs=sq[:, KT0:],
                                 start=(c == 0), stop=(c == NCH - 1))
            nc.scalar.activation(out=bias_sb[:, :KT0], in_=bias_ps[:],
                                 func=mybir.ActivationFunctionType.Copy, scale=-0.5)
            nc.scalar.activation(out=bias_sb[:, KT0:], in_=bias_ps2[:],
                                 func=mybir.ActivationFunctionType.Copy, scale=-0.5)

            # ---- G = centers @ W + b  -> g_dram ------------------------------
            for (m0, ms) in m_tiles:
                g_ps_full = main_psum.tile([P, K], dt.float32, tag="ps",
                                           name=f"g_ps{m0}{sfx}")
                g_ps = g_ps_full[:, :KT]
                if f32r_mode:
                    g_passes = [(cthi, wthi), (cthi, wtlo), (ctlo, wthi)]
                else:
                    g_passes = [(ctf, wt)]
                first = True
                for gi, (cs, ws) in enumerate(g_passes):
                    for c in range(NCH):
                        last = gi == len(g_passes) - 1 and c == NCH - 1
                        nc.tensor.matmul(out=g_ps[:ms, :], lhsT=cs[c][:][:, m0:m0 + ms],
                                         rhs=ws[c][:][:, :], start=first, stop=last)
                        first = False
                g_sb = gsb_pool.tile([P, KT], dt.float32, tag="g_sb", name=f"g_sb{m0}{sfx}")
                nc.vector.tensor_add(out=g_sb[:ms, :], in0=g_ps[:ms, :],
                                     in1=brep_sb[:ms, :])
                nc.scalar.dma_start(out=g_dram[m0:m0 + ms, :], in_=g_sb[:ms, :])

            # ---- main loop: scores -> argmax -> gather -----------------------
            for b in range(BH):
                if f32r_mode:
                    xhi, xlo = [], []
                    for c in range(NCH):
                        sl = slice(c * P, (c + 1) * P)
                        th = xt_pool.tile([P, S], dt.float32r, tag="xthi",
                                          name=f"xhi{b}_{c}{sfx}")
                        tl = xt_pool.tile([P, S], dt.float32r, tag="xtlo",
                                          name=f"xlo{b}_{c}{sfx}")
                        nc.sync.dma_start(out=th[:], in_=hT_hi[b, sl, :])
                        nc.sync.dma_start(out=tl[:], in_=hT_lo[b, sl, :])
                        xhi.append(th)
                        xlo.append(tl)
                    passes = [(xhi, cthi), (xhi, ctlo), (xlo, cthi)]
                else:
                    xts = []
                    for c in range(NCH):
                        xt = xt_pool.tile([P, S], dt.float32, tag="xthi",
                                          name=f"xt{b}_{c}{sfx}")
                        nc.sync.dma_start(out=xt[:], in_=hT_f[b, c * P:(c + 1) * P, :])
                        xts.append(xt)
                    passes = [(xts, ctf)]
                np_ = len(passes)
                for (s0, ns) in s_tiles:
                    ps = main_psum.tile([P, K], dt.float32, tag="ps", name=f"ps{b}_{s0}{sfx}")
                    first = True
                    for pi, (xs, cs) in enumerate(passes):
                        last_pass = pi == np_ - 1
                        for c in range(NCH):
                            last = last_pass and c == NCH - 1
                            nc.tensor.matmul(out=ps[:ns, :KT0],
                                             lhsT=xs[c][:][:, s0:s0 + ns],
                                             rhs=cs[c][:][:, :KT0],
                                             start=first, stop=last)
                            nc.tensor.matmul(out=ps[:ns, KT0:],
                                             lhsT=xs[c][:][:, s0:s0 + ns],
                                             rhs=cs[c][:][:, KT0:],
                                             start=first, stop=last)
                            first = False
                    sc = sc_pool.tile([P, K], dt.float32, tag="sc", name=f"sc{b}_{s0}{sfx}")
                    nc.vector.tensor_add(out=sc[:ns, :], in0=ps[:ns, :],
                                         in1=bias_sb[:ns, :])
                    max8 = idx_pool.tile([P, 8], dt.float32, tag="max8",
                                         name=f"max8{b}_{s0}{sfx}")
                    idx8 = idx_pool.tile([P, 8], dt.uint32, tag="idx8",
                                         name=f"idx8{b}_{s0}{sfx}")
                    nc.vector.max(out=max8[:ns, :], in_=sc[:ns, :])
                    nc.vector.max_index(out=idx8[:ns, :], in_max=max8[:ns, :],
                                        in_values=sc[:ns, :])
                    tok = idx_pool.tile([P, 1], dt.int32, tag="tok",
                                        name=f"tok{b}_{s0}{sfx}")
                    nc.vector.tensor_copy(out=tok[:ns, :], in_=idx8[:ns, :1])
                    nc.scalar.dma_start(out=toks_out[b, s0:s0 + ns, None], in_=tok[:ns, :])
                    emb = emb_pool.tile([P, KT], dt.float32, tag="emb", name=f"emb{b}_{s0}{sfx}")
                    nc.gpsimd.indirect_dma_start(
                        out=emb[:ns, :], out_offset=None,
                        in_=g_dram[:, :],
                        in_offset=IndirectOffsetOnAxis(ap=tok[:ns, :1], axis=0),
                    )
                    nc.scalar.dma_start(out=embs_out[b, s0:s0 + ns, :], in_=emb[:ns, :])

    nc.compile()
    return nc


def _get_program(score_mode=SCORE_MODE, reps=1):
    key = (score_mode, reps)
    if key not in _programs:
        _programs[key] = _build_program(score_mode, reps)
    return _programs[key]


def _make_in_maps(h, centers, W, b, score_mode=SCORE_MODE):
    in_maps = []
    for core in range(NCORES):
        l = core // 2
        bh = core % 2
        hT_core = np.ascontiguousarray(
            h[l, bh * BH:(bh + 1) * BH].reshape(BH, D, S)).astype(np.float32, copy=False)
        cT_core = np.ascontiguousarray(centers[l].T).astype(np.float32, copy=False)
        w_core = np.ascontiguousarray(W[l]).astype(np.float32, copy=False)
        brep_core = np.ascontiguousarray(
            np.broadcast_to(b[l][None, :], (P, KT))).astype(np.float32, copy=False)
        m = {"brep": brep_core}
        if score_mode == "f32r3":
            hhi = _round_f32r(hT_core)
            m["hT_hi"] = hhi
            m["hT_lo"] = _round_f32r(hT_core - hhi)
            chi = _round_f32r(cT_core)
            m["cT_hi"] = chi
            m["cT_lo"] = _round_f32r(cT_core - chi)
            whi = _round_f32r(w_core)
            m["w_hi"] = whi
            m["w_lo"] = _round_f32r(w_core - whi)
        else:
            m["hT"] = hT_core
            m["cT"] = cT_core
            m["w_in"] = w_core
        in_maps.append(m)
    return in_maps


class _Runner:
    """Device-resident SPMD runner mirroring bass2jax.run_bass_via_pjrt's
    multi-core path, but reusable across calls (for benchmarking)."""

    def __init__(self, nc, n_cores):
        import jax
        import concourse.mybir as mybir
        from concourse.bass2jax import _bass_exec_p, install_neuronx_cc_hook
        from jax.experimental.shard_map import shard_map
        from jax.sharding import Mesh, PartitionSpec

        install_neuronx_cc_hook()
        self.n_cores = n_cores
        in_names, out_names, out_avals, zero_outs = [], [], [], []
        for alloc in nc.m.functions[0].allocations:
            if not isinstance(alloc, mybir.MemoryLocationSet):
                continue
            name = alloc.memorylocations[0].name
            if alloc.kind == "ExternalInput":
                in_names.append(name)
            elif alloc.kind == "ExternalOutput":
                shape = tuple(alloc.tensor_shape)
                dtype = mybir.dt.np(alloc.dtype)
                out_avals.append(jax.core.ShapedArray(shape, dtype))
                zero_outs.append(np.zeros(shape, dtype))
                out_names.append(name)
        n_params = len(in_names)
        all_in_names = in_names + out_names
        self.in_names, self.out_names = in_names, out_names
        self.out_avals = out_avals

        def _body(*args):
            outs = _bass_exec_p.bind(
                *args,
                out_avals=tuple(out_avals),
                in_names=tuple(all_in_names),
                out_names=tuple(out_names),
                lowering_input_output_aliases=(),
                sim_require_finite=True,
                sim_require_nnan=True,
                nc=nc,
            )
            return tuple(outs)

        devices = jax.devices()[:n_cores]
        assert len(devices) == n_cores
        self.mesh = Mesh(np.asarray(devices), ("core",))
        in_specs = (PartitionSpec("core"),) * (n_params + len(out_names))
        out_specs = (PartitionSpec("core"),) * len(out_names)
        self.fn = jax.jit(
            shard_map(_body, mesh=self.mesh, in_specs=in_specs,
                      out_specs=out_specs, check_rep=False),
            keep_unused=True,
        )
        self.zero_outs = zero_outs
        self._dev_zeros = None

    def put_inputs(self, in_maps):
        import jax
        from jax.sharding import NamedSharding, PartitionSpec
        sh = NamedSharding(self.mesh, PartitionSpec("core"))
        concat_in = [
            np.concatenate([np.asarray(m[name]) for m in in_maps], axis=0)
            for name in self.in_names
        ]
        self._dev_in = [jax.device_put(a, sh) for a in concat_in]
        if self._dev_zeros is None:
            concat_zero = [
                np.zeros((self.n_cores * z.shape[0], *z.shape[1:]), z.dtype)
                for z in self.zero_outs
            ]
            self._dev_zeros = [jax.device_put(a, sh) for a in concat_zero]

    def run(self):
        import jax
        outs = self.fn(*self._dev_in, *self._dev_zeros)
        jax.block_until_ready(outs)
        return outs

    def results(self, outs):
        res = []
        for c in range(self.n_cores):
            res.append({
                name: np.asarray(outs[i]).reshape(
                    self.n_cores, *self.out_avals[i].shape)[c]
                for i, name in enumerate(self.out_names)
            })
        return res

    def timed(self, iters):
        import jax, time
        t0 = time.perf_counter()
        for _ in range(iters):
            outs = self.fn(*self._dev_in, *self._dev_zeros)
        jax.block_until_ready(outs)
        return time.perf_counter() - t0


def _get_runner(score_mode=SCORE_MODE, reps=1):
    key = (score_mode, reps)
    if key not in _runners:
        _runners[key] = _Runner(_get_program(score_mode, reps), NCORES)
    return _runners[key]


def kernel(h, centers, W, b):
    h = np.asarray(h, dtype=np.float32)
    centers = np.asarray(centers, dtype=np.float32)
    W = np.asarray(W, dtype=np.float32)
    b = np.asarray(b, dtype=np.float32)

    runner = _get_runner()
    in_maps = _make_in_maps(h, centers, W, b)
    runner.put_inputs(in_maps)
    outs = runner.run()
    res = runner.results(outs)

    tokens_full = np.empty((B, S, L), np.int32)
    embs_full = np.empty((B, S, L, KT), np.float32)
    for core in range(NCORES):
        l = core // 2
        bh = core % 2
        out = res[core]
        tokens_full[bh * BH:(bh + 1) * BH, :, l] = out["toks"]
        embs_full[bh * BH:(bh + 1) * BH, :, l, :] = out["embs"]
    return tokens_full, embs_full
